# revision 3
# baseline (speedup 1.0000x reference)
"""Trainium2 Bass kernel for nn_ANFISRuleStrengthLayer.

Math (N_INPUTS=2, N_MF=64): out[b, j1*64 + j2] = x[b, 0, j1] * x[b, 1, j2]
i.e. a per-sample outer product of two 64-vectors.
Input  x:   (32768, 2, 64) f32
Output out: (32768, 4096)  f32   (512 MiB -> heavily DMA-write bound)

Sharding: pure data parallel over batch across 8 NeuronCores
(4096 rows per core, no cross-core communication).
"""

import os
from contextlib import ExitStack

import numpy as np

import concourse.bass as bass
import concourse.mybir as mybir
import concourse.tile as tile
from concourse import bacc
from concourse.bass_utils import run_bass_kernel_spmd

BATCH = 32768
N_MF = 64
CONSEQ = N_MF * N_MF  # 4096
N_CORES = 8
SHARD = BATCH // N_CORES  # 4096
P = 128  # partitions


def build_kernel(tc: tile.TileContext, out_ap: bass.AP, x_ap: bass.AP, shard: int = SHARD):
    """Per-core kernel body. x_ap: [shard, 2, 64] f32, out_ap: [shard, 4096] f32."""
    nc = tc.nc
    n_tiles = shard // P
    with ExitStack() as ctx:
        in_pool = ctx.enter_context(tc.tile_pool(name="xin", bufs=1))
        out_pool = ctx.enter_context(tc.tile_pool(name="out", bufs=4))

        # One bulk load of the whole shard: [shard, 2, 64] -> SBUF [128, n_tiles*128]
        # xt[p, t*128 + i*64 + m] = x[t*128 + p, i, m]
        xt = in_pool.tile([P, n_tiles * 2 * N_MF], mybir.dt.float32)
        nc.sync.dma_start(
            xt[:].rearrange("p (t k) -> p t k", t=n_tiles),
            x_ap.rearrange("(t p) i m -> p t (i m)", p=P),
        )

        for t in range(n_tiles):
            a = xt[:, t * 128 : t * 128 + N_MF]           # x[rows, 0, :]
            b = xt[:, t * 128 + N_MF : t * 128 + 2 * N_MF]  # x[rows, 1, :]
            ot = out_pool.tile([P, CONSEQ], mybir.dt.float32)
            # out[p, j1, j2] = a[p, j1] * b[p, j2] in ONE fp32 tensor_tensor
            # via stride-0 broadcast access patterns.
            nc.vector.tensor_mul(
                ot[:].rearrange("p (a b) -> p a b", a=N_MF),
                a.unsqueeze(2).to_broadcast([P, N_MF, N_MF]),
                b.unsqueeze(1).to_broadcast([P, N_MF, N_MF]),
            )
            nc.sync.dma_start(out_ap[t * P : (t + 1) * P, :], ot[:])


def _ensure_trace_support():
    """Install the NTFF profile hook that the slim agent image omits.

    run_bass_kernel_spmd(trace=True) under axon imports
    antenv.axon_hooks.get_axon_ntff_profile_hook; the container's antenv
    stub lacks that module. Recreate it in sys.modules, backed by the
    ctypes hook factory in trn_agent_boot.trn_boot.
    """
    import sys
    import types

    if "antenv.axon_hooks" in sys.modules:
        return
    try:
        from trn_agent_boot.trn_boot import _ntff_profile_via_ctypes

        hook = _ntff_profile_via_ctypes("/opt/axon/libaxon_pjrt.so")
    except Exception:
        hook = None
    mod = types.ModuleType("antenv.axon_hooks")
    _state = {"hook": hook}
    mod.get_axon_ntff_profile_hook = lambda: _state["hook"]
    mod.set_axon_ntff_profile_hook = lambda h: _state.__setitem__("hook", h)
    sys.modules["antenv.axon_hooks"] = mod
    import antenv

    antenv.axon_hooks = mod


_CACHED = {}


def _build(shard: int = SHARD):
    key = shard
    if key in _CACHED:
        return _CACHED[key]
    nc = bacc.Bacc(
        "TRN2",
        target_bir_lowering=False,
        debug=False,
        enable_asserts=False,
        num_devices=N_CORES,
    )
    x_t = nc.dram_tensor("x", [shard, 2, N_MF], mybir.dt.float32, kind="ExternalInput")
    out_t = nc.dram_tensor("out", [shard, CONSEQ], mybir.dt.float32, kind="ExternalOutput")
    with tile.TileContext(nc) as tc:
        build_kernel(tc, out_t.ap(), x_t.ap(), shard)
    nc.compile()
    _CACHED[key] = nc
    return nc


def _run(x: np.ndarray, trace: bool = False):
    """Run on 8 cores. Returns (out [32768,4096] f32, BassKernelResults)."""
    x = np.ascontiguousarray(np.asarray(x, dtype=np.float32))
    assert x.shape == (BATCH, 2, N_MF), x.shape
    if trace:
        _ensure_trace_support()
    nc = _build()
    in_maps = [{"x": x[c * SHARD : (c + 1) * SHARD]} for c in range(N_CORES)]
    res = run_bass_kernel_spmd(nc, in_maps, core_ids=list(range(N_CORES)), trace=trace)
    out = np.concatenate([res.results[c]["out"] for c in range(N_CORES)], axis=0)
    return out, res


def kernel(**inputs: np.ndarray) -> np.ndarray:
    out, _ = _run(inputs["x"], trace=bool(int(os.environ.get("KERNEL_TRACE", "0"))))
    return out


# revision 4
# speedup vs baseline: 1.0341x; 1.0341x over previous
"""Trainium2 Bass kernel for nn_ANFISRuleStrengthLayer.

Math (N_INPUTS=2, N_MF=64): out[b, j1*64 + j2] = x[b, 0, j1] * x[b, 1, j2]
i.e. a per-sample outer product of two 64-vectors.
Input  x:   (32768, 2, 64) f32
Output out: (32768, 4096)  f32   (512 MiB -> heavily DMA-write bound)

Sharding: pure data parallel over batch across 8 NeuronCores
(4096 rows per core, no cross-core communication).
"""

import os
from contextlib import ExitStack

import numpy as np

import concourse.bass as bass
import concourse.mybir as mybir
import concourse.tile as tile
from concourse import bacc
from concourse.bass_utils import run_bass_kernel_spmd

BATCH = 32768
N_MF = 64
CONSEQ = N_MF * N_MF  # 4096
N_CORES = 8
SHARD = BATCH // N_CORES  # 4096
P = 128  # partitions


def build_kernel(tc: tile.TileContext, out_ap: bass.AP, x_ap: bass.AP, shard: int = SHARD):
    """Per-core kernel body. x_ap: [shard, 2, 64] f32, out_ap: [shard, 4096] f32."""
    nc = tc.nc
    n_tiles = shard // P
    with ExitStack() as ctx:
        in_pool = ctx.enter_context(tc.tile_pool(name="xin", bufs=1))
        out_pool = ctx.enter_context(tc.tile_pool(name="out", bufs=6))

        # Chunked load of the shard: [shard, 2, 64] -> SBUF [128, n_tiles*128]
        # xt[p, t*128 + i*64 + m] = x[t*128 + p, i, m]
        # First chunk is small so compute can start ASAP; input DMAs ride the
        # ACT HWDGE ring (nc.scalar) to stay out of the output ring's FIFO.
        xt = in_pool.tile([P, n_tiles * 2 * N_MF], mybir.dt.float32)
        xt3 = xt[:].rearrange("p (t k) -> p t k", t=n_tiles)
        xd3 = x_ap.rearrange("(t p) i m -> p t (i m)", p=P)
        chunks = [1, 1, 2, 4] + [4] * ((n_tiles - 8) // 4) if n_tiles >= 8 else [1] * n_tiles
        t0 = 0
        for c in chunks:
            nc.scalar.dma_start(xt3[:, t0 : t0 + c, :], xd3[:, t0 : t0 + c, :])
            t0 += c
        assert t0 == n_tiles

        for t in range(n_tiles):
            a = xt[:, t * 128 : t * 128 + N_MF]           # x[rows, 0, :]
            b = xt[:, t * 128 + N_MF : t * 128 + 2 * N_MF]  # x[rows, 1, :]
            ot = out_pool.tile([P, CONSEQ], mybir.dt.float32)
            # out[p, j1, j2] = a[p, j1] * b[p, j2] in ONE fp32 tensor_tensor
            # via stride-0 broadcast access patterns.
            nc.vector.tensor_mul(
                ot[:].rearrange("p (a b) -> p a b", a=N_MF),
                a.unsqueeze(2).to_broadcast([P, N_MF, N_MF]),
                b.unsqueeze(1).to_broadcast([P, N_MF, N_MF]),
            )
            nc.sync.dma_start(out_ap[t * P : (t + 1) * P, :], ot[:])


def _ensure_trace_support():
    """Install the NTFF profile hook that the slim agent image omits.

    run_bass_kernel_spmd(trace=True) under axon imports
    antenv.axon_hooks.get_axon_ntff_profile_hook; the container's antenv
    stub lacks that module. Recreate it in sys.modules, backed by the
    ctypes hook factory in trn_agent_boot.trn_boot.
    """
    import sys
    import types

    if "antenv.axon_hooks" in sys.modules:
        return
    try:
        from trn_agent_boot.trn_boot import _ntff_profile_via_ctypes

        hook = _ntff_profile_via_ctypes("/opt/axon/libaxon_pjrt.so")
    except Exception:
        hook = None
    mod = types.ModuleType("antenv.axon_hooks")
    _state = {"hook": hook}
    mod.get_axon_ntff_profile_hook = lambda: _state["hook"]
    mod.set_axon_ntff_profile_hook = lambda h: _state.__setitem__("hook", h)
    sys.modules["antenv.axon_hooks"] = mod
    import antenv

    antenv.axon_hooks = mod


_CACHED = {}


def _build(shard: int = SHARD):
    key = shard
    if key in _CACHED:
        return _CACHED[key]
    nc = bacc.Bacc(
        "TRN2",
        target_bir_lowering=False,
        debug=False,
        enable_asserts=False,
        num_devices=N_CORES,
    )
    x_t = nc.dram_tensor("x", [shard, 2, N_MF], mybir.dt.float32, kind="ExternalInput")
    out_t = nc.dram_tensor("out", [shard, CONSEQ], mybir.dt.float32, kind="ExternalOutput")
    with tile.TileContext(nc) as tc:
        build_kernel(tc, out_t.ap(), x_t.ap(), shard)
    nc.compile()
    _CACHED[key] = nc
    return nc


def _run(x: np.ndarray, trace: bool = False):
    """Run on 8 cores. Returns (out [32768,4096] f32, BassKernelResults)."""
    x = np.ascontiguousarray(np.asarray(x, dtype=np.float32))
    assert x.shape == (BATCH, 2, N_MF), x.shape
    if trace:
        _ensure_trace_support()
    nc = _build()
    in_maps = [{"x": x[c * SHARD : (c + 1) * SHARD]} for c in range(N_CORES)]
    res = run_bass_kernel_spmd(nc, in_maps, core_ids=list(range(N_CORES)), trace=trace)
    out = np.concatenate([res.results[c]["out"] for c in range(N_CORES)], axis=0)
    return out, res


def kernel(**inputs: np.ndarray) -> np.ndarray:
    out, _ = _run(inputs["x"], trace=bool(int(os.environ.get("KERNEL_TRACE", "0"))))
    return out
